# revision 68
# baseline (speedup 1.0000x reference)
"""Trainium2 Bass kernel for GQA attention (B=2, S=2048, DIM=4096, 32 q-heads,
8 kv-heads, head_dim=128, RoPE, causal).

Sharding: 8 cores = (2 batches) x (4 kv-head groups of 2 kv-heads / 8 q-heads).
No collectives: each core emits a partial (wo row-sharded) output; the host
sums the 4 group-partials per batch.

Per-core layouts (everything stays "transposed", head_dim/feature on
partitions, tokens on the free axis) so no on-chip transposes are needed:
  xT  [4096, S]      QK-proj:   QT/KT = Wqk^T @ xT     (lhsT = Wqk, rhs = xT)
  V   [S, 256]       V-proj:    V = xT^T @ Wv          (lhsT = xT,  rhs = Wv)
  S^T [kv, q]        scores:    lhsT = KT tile, rhs = QT
  P^T = exp(S^T)     (scores ~ N(0,1): softmax without max subtraction)
  OUT^T [d, q]       PV:        lhsT = V tile,  rhs = P^T
  sums [1, q]        DVE bf16 group-accumulate of P^T tiles (4 kv-tiles per
                     group), then one PE ones-matmul per group (PSUM f32
                     accumulation across groups) — 4x fewer ones-matmul
                     columns than per-tile sums; the normalization 1/sums is
                     applied to OUT^T via gpsimd partition_broadcast + DVE
                     multiply.
  final^T [4096, q]  wo-proj:   lhsT = wo tile, rhs = OUT^T

Software pipeline: attention for q-block j is emitted directly after the
projections+RoPE of token block tb=j (causality: it only needs k/v blocks
<= j), and the wo matmuls for block j-1 are interleaved into attention j's
k-loops.  This keeps ACT (exp), DVE (rope/copies/sums) and Pool (reduces)
running concurrently with the PE stream at every point of the kernel.

RoPE: wq/wk columns are permuted per head on the host (even dims first, odd
dims second) so rotation becomes the "rotate-half" form; scores are invariant
under a shared permutation of q/k head dims. 1/sqrt(HD) is folded into wq.
"""

import numpy as np
import ml_dtypes

BF16 = ml_dtypes.bfloat16

B, S_FULL, DIM = 2, 2048, 4096
NH, NKV, HD = 32, 8, 128
NREP = NH // NKV
NCORES = 8
NGRP = 4            # head groups
NQH = NH // NGRP    # 8 q heads per core
NKVH = NKV // NGRP  # 2 kv heads per core
NKT = DIM // 128    # 32 contraction tiles
TB = 512            # token block
NCT = (NQH * HD + NKVH * HD) // 128  # 10 col tiles (8 q + 2 k)

_cache = {}


def _build(S, reps=1, **opt):
    """Build + bacc-compile the per-core Bass module (same program on all 8)."""
    import concourse.mybir as mybir
    import concourse.tile as tile
    from concourse import bacc

    f32 = mybir.dt.float32
    bf16 = mybir.dt.bfloat16
    Exp = mybir.ActivationFunctionType.Exp
    mult = mybir.AluOpType.mult

    NT = S // TB          # token blocks
    NJ = S // TB          # q blocks
    NDT = DIM // 128

    nc = bacc.Bacc("TRN2", target_bir_lowering=False, debug=False,
                   num_devices=NCORES)

    xt_d = nc.dram_tensor("xt", [NT, 128, NKT, TB], bf16, kind="ExternalInput")
    wqk_d = nc.dram_tensor("wqk", [NCT, 128, NKT, 128], bf16, kind="ExternalInput")
    wv_d = nc.dram_tensor("wv", [128, NKT, NKVH * HD], bf16, kind="ExternalInput")
    wo_d = nc.dram_tensor("wo", [NDT, 128, NQH, 128], bf16, kind="ExternalInput")
    cosf_d = nc.dram_tensor("cosf", [128, S], bf16, kind="ExternalInput")
    sinf_d = nc.dram_tensor("sinf", [128, S], bf16, kind="ExternalInput")
    pat_d = nc.dram_tensor("pat", [128, 4, TB], bf16, kind="ExternalInput")
    ones_d = nc.dram_tensor("ones", [128, 1], bf16, kind="ExternalInput")
    ones2_d = nc.dram_tensor("ones2", [1, 128], bf16, kind="ExternalInput")
    out_d = nc.dram_tensor("outp", [NDT, 128, S], bf16, kind="ExternalOutput")

    with tile.TileContext(nc) as tc:
        from contextlib import ExitStack
        with ExitStack() as ctx:
            const_p = ctx.enter_context(tc.tile_pool(name="const", bufs=1))
            xt_p = ctx.enter_context(tc.tile_pool(name="xt", bufs=4))
            wqk_p = ctx.enter_context(tc.tile_pool(name="wqk", bufs=3))
            wo_p = ctx.enter_context(tc.tile_pool(name="wo", bufs=5))
            qt_p = ctx.enter_context(tc.tile_pool(name="qt", bufs=1))
            kt_p = ctx.enter_context(tc.tile_pool(name="kt", bufs=1))
            vt_p = ctx.enter_context(tc.tile_pool(name="vt", bufs=1))
            osb_p = ctx.enter_context(tc.tile_pool(name="osb", bufs=1))
            pt_p = ctx.enter_context(tc.tile_pool(name="pt", bufs=8))
            tmp_p = ctx.enter_context(tc.tile_pool(name="tmp", bufs=2))
            ga_p = ctx.enter_context(tc.tile_pool(name="ga", bufs=4))
            st_p = ctx.enter_context(tc.tile_pool(name="st", bufs=4))
            rr_p = ctx.enter_context(tc.tile_pool(name="rr", bufs=4))
            rbs_p = ctx.enter_context(tc.tile_pool(name="rbs", bufs=2))
            psA = ctx.enter_context(tc.tile_pool(name="psA", bufs=opt.get("psA", 3), space="PSUM"))
            psO = ctx.enter_context(tc.tile_pool(name="psO", bufs=opt.get("psO", 2), space="PSUM"))
            psS = ctx.enter_context(tc.tile_pool(name="psS", bufs=1, space="PSUM"))
            psW = ctx.enter_context(tc.tile_pool(name="psW", bufs=opt.get("psW", 2), space="PSUM"))

            # constants loaded once
            cosf = const_p.tile([128, S], bf16, tag="cosf")
            sinf = const_p.tile([128, S], bf16, tag="sinf")
            warm = const_p.tile([1, 8], f32, tag="warm")
            pat = const_p.tile([128, 4, TB], bf16, tag="pat")
            ones = const_p.tile([128, 1], bf16, tag="ones")
            ones2 = const_p.tile([1, 128], bf16, tag="ones2")
            wv = const_p.tile([128, NKT, NKVH * HD], bf16, tag="wv")

            NCH = 4            # xt split into 4 chunk tiles (parallel DMA)
            KCH = NKT // NCH   # k-tiles per chunk

            for _rep in range(reps):
                qt = qt_p.tile([128, NQH, S], bf16, tag="qt")
                pmall = psS.tile([128, TB], f32, tag="pmall")
                kt = kt_p.tile([128, NKVH, S], bf16, tag="kt")
                pending = [None]   # deferred normalization of the last pair

                def emit_norm():
                    if pending[0] is None:
                        return
                    pj, phh, ppo, prrs = pending[0]
                    pending[0] = None
                    for i, h in enumerate(phh):
                        # copy po to SBUF first (frees its PSUM bank, and a
                        # DVE op may read at most one PSUM operand), then
                        # rank-1 PE broadcast: ones2^T @ rr -> [128, TB]
                        pos = rbs_p.tile([128, TB], bf16, tag="rbs")
                        nc.vector.tensor_copy(pos[:], ppo[i][:])
                        rb = psW.tile([128, TB], f32, tag="pw",
                                      name=f"rb{pj}_{h}")
                        nc.tensor.matmul(rb[:], ones2[:], prrs[i][:],
                                         start=True, stop=True)
                        nc.vector.tensor_tensor(
                            osb[:, h, pj * TB:(pj + 1) * TB],
                            pos[:], rb[:], mult)
                vt = vt_p.tile([128, S // 128, NKVH * HD], bf16, tag="vt")
                osb = osb_p.tile([128, NQH, S], bf16, tag="osb")

                # ---- wo block helper (final projection, row-shard partial) --
                def wo_block(jw, Dt):
                    wo = wo_p.tile([128, NQH, 128], bf16, tag="wo",
                                   name=f"wo{jw}_{Dt}")
                    nc.sync.dma_start(wo[:], wo_d[Dt])
                    pw = psW.tile([128, TB], f32, tag="pw", name=f"pw{jw}_{Dt}")
                    for dt in range(NQH):
                        nc.tensor.matmul(
                            pw[:], wo[:, dt, :],
                            osb[:, dt, jw * TB:(jw + 1) * TB],
                            start=(dt == 0), stop=(dt == NQH - 1))
                    stg = st_p.tile([128, TB], bf16, tag="st",
                                    name=f"st{jw}_{Dt}")
                    nc.scalar.copy(stg[:], pw[:])
                    # store from the ACT DGE ring: keeps the SP ring free
                    # for weight/x loads (SP HWDGE gen is ~625ns per DMA)
                    nc.scalar.dma_start(out_d[Dt, :, jw * TB:(jw + 1) * TB],
                                        stg[:])

                def do_ct(ci, ct, tb, xch, w01, first):
                    if first and ci < 2:
                        w = w01[ci]
                    else:
                        w = wqk_p.tile([128, NKT, 128], bf16, tag="wqk")
                        nc.sync.dma_start(w[:], wqk_d[ct])
                    ps = psA.tile([128, TB], f32, tag="ps")
                    for k in range(NKT):
                        nc.tensor.matmul(
                            ps[:], w[:, k, :], xch[k // KCH][:, k % KCH, :],
                            start=(k == 0), stop=(k == NKT - 1))
                    if ct < NQH:
                        dst = qt[:, ct, tb * TB:(tb + 1) * TB]
                    else:
                        dst = kt[:, ct - NQH, tb * TB:(tb + 1) * TB]
                    nc.vector.tensor_copy(dst, ps[:])
                    # RoPE on this slice (DVE, overlaps next ct matmuls)
                    cs = cosf[:, tb * TB:(tb + 1) * TB]
                    sn = sinf[:, tb * TB:(tb + 1) * TB]
                    tmp = tmp_p.tile([128, TB], bf16, tag="tmp")
                    nc.vector.tensor_copy(tmp[0:64, :], dst[64:128, :])
                    nc.vector.tensor_copy(tmp[64:128, :], dst[0:64, :])
                    nc.vector.tensor_tensor(dst, dst, cs, mult)
                    nc.vector.tensor_tensor(tmp[:], tmp[:], sn, mult)
                    nc.vector.tensor_add(dst, dst, tmp[:])

                def do_v(st0, st1, tb, xch, first):
                    # V projection ([tok, d] layout)
                    if first and st0 == 0:
                        nc.sync.dma_start(wv[:], wv_d[:])
                    for st in range(st0, st1):
                        ps = psA.tile([128, NKVH * HD], f32, tag="ps")
                        for k in range(NKT):
                            nc.tensor.matmul(
                                ps[:],
                                xch[k // KCH][:, k % KCH, st * 128:(st + 1) * 128],
                                wv[:, k, :],
                                start=(k == 0), stop=(k == NKT - 1))
                        nc.vector.tensor_copy(vt[:, tb * (TB // 128) + st, :],
                                              ps[:])

                def attn_pair(j, pi):
                    nkv = (j + 1) * (TB // 128)
                    kvh = (2 * pi) // NREP
                    hh = (2 * pi, 2 * pi + 1)
                    po = []
                    for _i in range(2):
                        po_t = psO.tile([128, TB], f32, tag="po",
                                        name=f"po{_i}")
                        po.append(po_t)
                    gac = [None, None]
                    rrs = [None, None]

                    def emit_pv(pk, poff, pN, pts):
                        for i in range(2):
                            nc.tensor.matmul(
                                po[i][:, poff:],
                                vt[:, pk, kvh * HD:(kvh + 1) * HD],
                                pts[i][:, :pN],
                                start=(pk == 0), stop=(pk == nkv - 1),
                                skip_group_check=True)

                    pend = None
                    for k in range(nkv):
                        kd = k - j * (TB // 128)
                        off = max(0, kd) * 128   # causal col offset
                        N = TB - off
                        GSZ = 16                 # kv-tiles per sum group
                        gk = k % GSZ
                        glast = (gk == GSZ - 1) or (k == nkv - 1)
                        cur = []
                        for i, h in enumerate(hh):
                            # during attn(0) no wo blocks run, psW is idle:
                            # use it for h1 scores to relieve the psA cycle
                            if j == 0 and i == 1:
                                ss = psW.tile([128, TB], f32, tag="pw",
                                              name=f"ssb{pi}_{k}")
                            else:
                                ss = psA.tile([128, TB], f32, tag="ps")
                            nc.tensor.matmul(
                                ss[:, :N],
                                kt[:, kvh, k * 128:(k + 1) * 128],
                                qt[:, h, j * TB + off:(j + 1) * TB],
                                start=True, stop=True)
                            pt = pt_p.tile([128, TB], bf16, tag="pt")
                            nc.scalar.activation(pt[:, :N], ss[:, :N], Exp)
                            if kd >= 0:
                                # only the first 128 cols of the slice hold
                                # the causal triangle; the rest is all-ones
                                nc.vector.tensor_tensor(
                                    pt[:, :128], pt[:, :128],
                                    pat[:, kd, off:off + 128], mult)
                            cur.append(pt)
                            # ---- P row-sums: bf16 group accumulate on
                            # DVE, one ones-matmul per group of 4 ----
                            if gk == 0:
                                ga = ga_p.tile([128, TB], bf16, tag="ga")
                                gac[i] = ga
                                nc.vector.tensor_copy(ga[:, :N], pt[:, :N])
                            else:
                                nc.vector.tensor_add(
                                    gac[i][:, off:], gac[i][:, off:],
                                    pt[:, :N])
                        # depth-1 software pipeline: PV for k-1 lands
                        # after scores k, covering the exp+mask latency
                        if pend is not None:
                            emit_pv(*pend)
                        if k == 1:
                            emit_norm()   # previous pair, PE-covered
                        if glast:
                            for i in range(2):
                                nc.tensor.matmul(
                                    pmall[64 * i:64 * i + 1, :],
                                    ones[:], gac[i][:],
                                    start=(k < GSZ), stop=(k == nkv - 1),
                                    tile_position=(0, 64 * i),
                                    skip_group_check=True)
                            if k == nkv - 1:
                                # reciprocals ASAP so the rank-1
                                # broadcast matmuls aren't rr-blocked
                                for i in range(2):
                                    rr = rr_p.tile([1, TB], bf16,
                                                   tag="rr",
                                                   name=f"rr{pi}_{i}")
                                    rrs[i] = rr
                                    with nc.allow_low_precision(
                                            reason="1/sums bf16"):
                                        nc.vector.reciprocal(
                                            rr[:],
                                            pmall[64 * i:64 * i + 1, :])
                        # interleave wo fill work inside the k loop
                        if j > 0:
                            d0 = NDT * pi // (NQH // 2)
                            d1 = NDT * (pi + 1) // (NQH // 2)
                            nblk = d1 - d0
                            lo = nblk * k // nkv
                            hi = nblk * (k + 1) // nkv
                            for Dt in range(d0 + lo, d0 + hi):
                                wo_block(j - 1, Dt)
                        pend = (k, off, N, cur)
                    emit_pv(*pend)
                    pending[0] = (j, hh, po, rrs)

                for tb in range(NT):
                    # ---- projections + RoPE for token block tb, with the
                    # previous block's attention interleaved between units --
                    first = (_rep == 0 and tb == 0)
                    xch = []
                    w01 = [None, None]
                    if first:
                        # startup-optimized DMA order: feed the PE ASAP,
                        # defer constants it doesn't need yet
                        w0 = wqk_p.tile([128, NKT, 128], bf16, tag="wqk",
                                        name="w_first")
                        xc0 = xt_p.tile([128, KCH, TB], bf16, tag="xt")
                        NQ4 = NKT // 4
                        for q in range(4):
                            nc.sync.dma_start(
                                w0[:, q * NQ4:(q + 1) * NQ4, :],
                                wqk_d[NQH, :, q * NQ4:(q + 1) * NQ4, :])
                        nc.sync.dma_start(xc0[:, 0:KCH // 2, :],
                                          xt_d[0, :, 0:KCH // 2, :])
                        nc.sync.dma_start(xc0[:, KCH // 2:KCH, :],
                                          xt_d[0, :, KCH // 2:KCH, :])
                        w01[0] = w0
                        xch.append(xc0)
                        xc1 = xt_p.tile([128, KCH, TB], bf16, tag="xt")
                        nc.sync.dma_start(xc1[:], xt_d[0, :, KCH:2 * KCH, :])
                        xch.append(xc1)
                        w1 = wqk_p.tile([128, NKT, 128], bf16, tag="wqk",
                                        name="w_second")
                        nc.sync.dma_start(w1[:, 0:NKT // 2, :],
                                          wqk_d[NQH + 1, :, 0:NKT // 2, :])
                        nc.sync.dma_start(w1[:, NKT // 2:, :],
                                          wqk_d[NQH + 1, :, NKT // 2:, :])
                        w01[1] = w1
                        for ch in range(2, NCH):
                            xc = xt_p.tile([128, KCH, TB], bf16, tag="xt")
                            nc.sync.dma_start(
                                xc[:], xt_d[0, :, ch * KCH:(ch + 1) * KCH, :])
                            xch.append(xc)
                        nc.sync.dma_start(cosf[:], cosf_d[:])
                        nc.sync.dma_start(sinf[:], sinf_d[:])
                    else:
                        for ch in range(NCH):
                            xc = xt_p.tile([128, KCH, TB], bf16, tag="xt")
                            nc.sync.dma_start(
                                xc[:], xt_d[tb, :, ch * KCH:(ch + 1) * KCH, :])
                            xch.append(xc)

                    # pending norm of the previous attention's last pair
                    # must land before any wo block that reads its osb rows
                    emit_norm()

                    units = []
                    for ci, ct in enumerate([NQH, NQH + 1, 0, 1]):
                        units.append(
                            lambda ci=ci, ct=ct: do_ct(ci, ct, tb, xch,
                                                       w01, first))
                    units.append(lambda: do_v(0, 2, tb, xch, first))
                    units.append(lambda: do_v(2, 4, tb, xch, first))
                    for ci, ct in enumerate(range(2, NQH)):
                        units.append(
                            lambda ci=ci, ct=ct: do_ct(ci + 6, ct, tb, xch,
                                                       w01, first))

                    if tb == 0:
                        if first:
                            # pre-load the Exp activation table during the
                            # startup DMA window (LoadActFuncSet is ~1.3us)
                            nc.gpsimd.memset(warm[:], 0.0)
                            nc.scalar.activation(warm[:], warm[:], Exp)
                        for u in units:
                            u()
                        if first:
                            nc.sync.dma_start(pat[:], pat_d[:])
                            nc.sync.dma_start(ones[:], ones_d[:])
                            nc.sync.dma_start(ones2[:], ones2_d[:])
                    else:
                        # interleave attention(tb-1) pairs between proj units
                        seq = [[0, 1], [2, 3, 4], [5, 6, 7], [8, 9, 10], [11]]
                        for pi in range(NQH // 2):
                            for ui in seq[pi]:
                                units[ui]()
                            attn_pair(tb - 1, pi)
                        for ui in seq[NQH // 2]:
                            units[ui]()

                # ---- last attention block (wo(NT-2) interleaved inside) ----
                for pi in range(NQH // 2):
                    attn_pair(NT - 1, pi)

                # ---- tail: wo for the last q block ----
                emit_norm()
                for Dt in range(NDT):
                    wo_block(NJ - 1, Dt)

    nc.compile()
    return nc


_PERM = None


def _prep_core_inputs(x, freqs_cis, mask, wq, wk, wv, wo, b, g, S):
    """Host-side shard/permute/prepack for core (batch b, group g)."""
    global _PERM
    if _PERM is None or len(_PERM) != HD:
        _PERM = np.concatenate([np.arange(0, HD, 2), np.arange(1, HD, 2)])
    perm = _PERM
    NT = S // TB

    qh0 = g * NQH            # first q head
    kh0 = g * NKVH           # first kv head

    wq_g = wq[:, qh0 * HD:(qh0 + NQH) * HD].reshape(DIM, NQH, HD)[:, :, perm]
    wq_g = (wq_g * np.float32(HD ** -0.5)).reshape(DIM, NQH * HD)
    wk_g = wk[:, kh0 * HD:(kh0 + NKVH) * HD].reshape(DIM, NKVH, HD)[:, :, perm]
    wk_g = wk_g.reshape(DIM, NKVH * HD)
    wqk = np.concatenate([wq_g, wk_g], axis=1)          # [DIM, 1280]
    wqk = np.ascontiguousarray(
        wqk.reshape(NKT, 128, NCT, 128).transpose(2, 1, 0, 3)).astype(BF16)

    wv_g = wv[:, kh0 * HD:(kh0 + NKVH) * HD]            # [DIM, 256]
    wv_g = np.ascontiguousarray(
        wv_g.reshape(NKT, 128, NKVH * HD).transpose(1, 0, 2)).astype(BF16)

    wo_g = wo[qh0 * HD:(qh0 + NQH) * HD, :]             # [1024, DIM]
    wo_g = np.ascontiguousarray(
        wo_g.reshape(NQH, 128, DIM // 128, 128).transpose(2, 1, 0, 3)).astype(BF16)

    xb = x[b, :S, :]                                    # [S, DIM]
    xt = np.ascontiguousarray(
        xb.reshape(NT, TB, NKT, 128).transpose(0, 3, 2, 1)).astype(BF16)

    cos = freqs_cis[:S, :, 0]                           # [S, 64]
    sin = freqs_cis[:S, :, 1]
    cosf = np.ascontiguousarray(np.concatenate([cos, cos], 1).T).astype(BF16)
    sinf = np.ascontiguousarray(np.concatenate([-sin, sin], 1).T).astype(BF16)

    sub = mask[:TB, :TB]                                # [q, kv]
    pat = (sub.T.reshape(4, 128, TB) >= -0.5).astype(BF16)
    pat = np.ascontiguousarray(pat.transpose(1, 0, 2))

    ones = np.ones((128, 1), dtype=BF16)
    ones2 = np.ones((1, 128), dtype=BF16)
    return {"xt": xt, "wqk": wqk, "wv": wv_g, "wo": wo_g,
            "cosf": cosf, "sinf": sinf, "pat": pat, "ones": ones,
            "ones2": ones2}


def run(x, freqs_cis, mask, wq, wk, wv, wo, S=S_FULL, reps=1, time_it=False):
    from concourse.bass_utils import run_bass_kernel_spmd

    key = (S, reps)
    if key not in _cache:
        _cache[key] = _build(S, reps)
    nc = _cache[key]

    in_maps = []
    for c in range(NCORES):
        b, g = c // NGRP, c % NGRP
        in_maps.append(_prep_core_inputs(x, freqs_cis, mask, wq, wk, wv, wo,
                                         b, g, S))
    res = run_bass_kernel_spmd(nc, in_maps, core_ids=list(range(NCORES)))

    out = np.zeros((B, S, DIM), dtype=np.float32)
    for c in range(NCORES):
        b = c // NGRP
        pt = np.asarray(res.results[c]["outp"],
                        dtype=np.float32).reshape(DIM, S)  # partial final^T
        out[b] += pt.T
    return out


def kernel(x, start_pos, freqs_cis, mask, wq, wk, wv, wo):
    x = np.asarray(x, dtype=np.float32)
    freqs_cis = np.asarray(freqs_cis, dtype=np.float32)
    mask = np.asarray(mask, dtype=np.float32)
    wq = np.asarray(wq, dtype=np.float32)
    wk = np.asarray(wk, dtype=np.float32)
    wv = np.asarray(wv, dtype=np.float32)
    wo = np.asarray(wo, dtype=np.float32)
    return run(x, freqs_cis, mask, wq, wk, wv, wo, S=x.shape[1], reps=1)


# revision 69
# speedup vs baseline: 1.3931x; 1.3931x over previous
"""Trainium2 Bass kernel for GQA attention (B=2, S=2048, DIM=4096, 32 q-heads,
8 kv-heads, head_dim=128, RoPE, causal).

Sharding: 8 cores = (2 batches) x (4 kv-head groups of 2 kv-heads / 8 q-heads).
No collectives: each core emits a partial (wo row-sharded) output; the host
sums the 4 group-partials per batch.

Per-core layouts (everything stays "transposed", head_dim/feature on
partitions, tokens on the free axis) so no on-chip transposes are needed:
  xT  [4096, S]      QK-proj:   QT/KT = Wqk^T @ xT     (lhsT = Wqk, rhs = xT)
  V   [S, 256]       V-proj:    V = xT^T @ Wv          (lhsT = xT,  rhs = Wv)
  S^T [kv, q]        scores:    lhsT = KT tile, rhs = QT
  P^T = exp(S^T)     (scores ~ N(0,1): softmax without max subtraction)
  OUT^T [d, q]       PV:        lhsT = V tile,  rhs = P^T
  sums [1, q]        DVE bf16 group-accumulate of P^T tiles (4 kv-tiles per
                     group), then one PE ones-matmul per group (PSUM f32
                     accumulation across groups) — 4x fewer ones-matmul
                     columns than per-tile sums; the normalization 1/sums is
                     applied to OUT^T via gpsimd partition_broadcast + DVE
                     multiply.
  final^T [4096, q]  wo-proj:   lhsT = wo tile, rhs = OUT^T

Software pipeline: attention for q-block j is emitted directly after the
projections+RoPE of token block tb=j (causality: it only needs k/v blocks
<= j), and the wo matmuls for block j-1 are interleaved into attention j's
k-loops.  This keeps ACT (exp), DVE (rope/copies/sums) and Pool (reduces)
running concurrently with the PE stream at every point of the kernel.

RoPE: wq/wk columns are permuted per head on the host (even dims first, odd
dims second) so rotation becomes the "rotate-half" form; scores are invariant
under a shared permutation of q/k head dims. 1/sqrt(HD) is folded into wq.
"""

import numpy as np
import ml_dtypes

BF16 = ml_dtypes.bfloat16

B, S_FULL, DIM = 2, 2048, 4096
NH, NKV, HD = 32, 8, 128
NREP = NH // NKV
NCORES = 8
NGRP = 4            # head groups
NQH = NH // NGRP    # 8 q heads per core
NKVH = NKV // NGRP  # 2 kv heads per core
NKT = DIM // 128    # 32 contraction tiles
TB = 512            # token block
NCT = (NQH * HD + NKVH * HD) // 128  # 10 col tiles (8 q + 2 k)

_cache = {}


def _build(S, reps=1, **opt):
    """Build + bacc-compile the per-core Bass module (same program on all 8)."""
    import concourse.mybir as mybir
    import concourse.tile as tile
    from concourse import bacc

    f32 = mybir.dt.float32
    bf16 = mybir.dt.bfloat16
    Exp = mybir.ActivationFunctionType.Exp
    mult = mybir.AluOpType.mult

    NT = S // TB          # token blocks
    NJ = S // TB          # q blocks
    NDT = DIM // 128

    nc = bacc.Bacc("TRN2", target_bir_lowering=False, debug=False,
                   num_devices=NCORES)

    xt_d = nc.dram_tensor("xt", [NT, 128, NKT, TB], bf16, kind="ExternalInput")
    wqk_d = nc.dram_tensor("wqk", [NCT, 128, NKT, 128], bf16, kind="ExternalInput")
    wv_d = nc.dram_tensor("wv", [128, NKT, NKVH * HD], bf16, kind="ExternalInput")
    wo_d = nc.dram_tensor("wo", [NDT, 128, NQH, 128], bf16, kind="ExternalInput")
    cosf_d = nc.dram_tensor("cosf", [128, S], bf16, kind="ExternalInput")
    sinf_d = nc.dram_tensor("sinf", [128, S], bf16, kind="ExternalInput")
    pat_d = nc.dram_tensor("pat", [128, 4, TB], bf16, kind="ExternalInput")
    ones_d = nc.dram_tensor("ones", [128, 1], bf16, kind="ExternalInput")
    ones2_d = nc.dram_tensor("ones2", [1, 128], bf16, kind="ExternalInput")
    out_d = nc.dram_tensor("outp", [NDT, 128, S], bf16, kind="ExternalOutput")

    with tile.TileContext(nc) as tc:
        from contextlib import ExitStack
        with ExitStack() as ctx:
            const_p = ctx.enter_context(tc.tile_pool(name="const", bufs=1))
            xt_p = ctx.enter_context(tc.tile_pool(name="xt", bufs=4))
            wqk_p = ctx.enter_context(tc.tile_pool(name="wqk", bufs=3))
            wo_p = ctx.enter_context(tc.tile_pool(name="wo", bufs=5))
            qt_p = ctx.enter_context(tc.tile_pool(name="qt", bufs=1))
            kt_p = ctx.enter_context(tc.tile_pool(name="kt", bufs=1))
            vt_p = ctx.enter_context(tc.tile_pool(name="vt", bufs=1))
            osb_p = ctx.enter_context(tc.tile_pool(name="osb", bufs=1))
            pt_p = ctx.enter_context(tc.tile_pool(name="pt", bufs=8))
            tmp_p = ctx.enter_context(tc.tile_pool(name="tmp", bufs=2))
            ga_p = ctx.enter_context(tc.tile_pool(name="ga", bufs=4))
            st_p = ctx.enter_context(tc.tile_pool(name="st", bufs=4))
            rr_p = ctx.enter_context(tc.tile_pool(name="rr", bufs=4))
            rbs_p = ctx.enter_context(tc.tile_pool(name="rbs", bufs=2))
            psA = ctx.enter_context(tc.tile_pool(name="psA", bufs=opt.get("psA", 3), space="PSUM"))
            psO = ctx.enter_context(tc.tile_pool(name="psO", bufs=opt.get("psO", 2), space="PSUM"))
            psS = ctx.enter_context(tc.tile_pool(name="psS", bufs=1, space="PSUM"))
            psW = ctx.enter_context(tc.tile_pool(name="psW", bufs=opt.get("psW", 2), space="PSUM"))

            # constants loaded once
            cosf = const_p.tile([128, S], bf16, tag="cosf")
            sinf = const_p.tile([128, S], bf16, tag="sinf")
            warm = const_p.tile([1, 8], f32, tag="warm")
            pat = const_p.tile([128, 4, TB], bf16, tag="pat")
            ones = const_p.tile([128, 1], bf16, tag="ones")
            ones2 = const_p.tile([1, 128], bf16, tag="ones2")
            wv = const_p.tile([128, NKT, NKVH * HD], bf16, tag="wv")

            NCH = 4            # xt split into 4 chunk tiles (parallel DMA)
            KCH = NKT // NCH   # k-tiles per chunk

            for _rep in range(reps):
                qt = qt_p.tile([128, NQH, S], bf16, tag="qt")
                pmall = psS.tile([128, TB], f32, tag="pmall")
                kt = kt_p.tile([128, NKVH, S], bf16, tag="kt")
                pending = [None]   # deferred normalization of the last pair

                def emit_norm():
                    if pending[0] is None:
                        return
                    pj, phh, ppo, prrs = pending[0]
                    pending[0] = None
                    for i, h in enumerate(phh):
                        # 1/sums broadcast on the (otherwise idle) Pool
                        # engine; the deferred-norm slack (a full pair)
                        # hides its ~2.5us real ucode latency.  rb lands in
                        # SBUF so the osb multiply reads only one PSUM
                        # operand (po).
                        rb = rbs_p.tile([128, TB], bf16, tag="rbs")
                        nc.gpsimd.partition_broadcast(rb[:], prrs[i][:])
                        nc.vector.tensor_tensor(
                            osb[:, h, pj * TB:(pj + 1) * TB],
                            ppo[i][:], rb[:], mult)
                vt = vt_p.tile([128, S // 128, NKVH * HD], bf16, tag="vt")
                osb = osb_p.tile([128, NQH, S], bf16, tag="osb")

                # ---- wo block helper (final projection, row-shard partial) --
                def wo_block(jw, Dt):
                    wo = wo_p.tile([128, NQH, 128], bf16, tag="wo",
                                   name=f"wo{jw}_{Dt}")
                    nc.sync.dma_start(wo[:], wo_d[Dt])
                    pw = psW.tile([128, TB], f32, tag="pw", name=f"pw{jw}_{Dt}")
                    for dt in range(NQH):
                        nc.tensor.matmul(
                            pw[:], wo[:, dt, :],
                            osb[:, dt, jw * TB:(jw + 1) * TB],
                            start=(dt == 0), stop=(dt == NQH - 1))
                    stg = st_p.tile([128, TB], bf16, tag="st",
                                    name=f"st{jw}_{Dt}")
                    nc.scalar.copy(stg[:], pw[:])
                    # store from the ACT DGE ring: keeps the SP ring free
                    # for weight/x loads (SP HWDGE gen is ~625ns per DMA)
                    nc.scalar.dma_start(out_d[Dt, :, jw * TB:(jw + 1) * TB],
                                        stg[:])

                def do_ct(ci, ct, tb, xch, w01, first):
                    if first and ci < 2:
                        w = w01[ci]
                    else:
                        w = wqk_p.tile([128, NKT, 128], bf16, tag="wqk")
                        nc.sync.dma_start(w[:], wqk_d[ct])
                    ps = psA.tile([128, TB], f32, tag="ps")
                    for k in range(NKT):
                        nc.tensor.matmul(
                            ps[:], w[:, k, :], xch[k // KCH][:, k % KCH, :],
                            start=(k == 0), stop=(k == NKT - 1))
                    if ct < NQH:
                        dst = qt[:, ct, tb * TB:(tb + 1) * TB]
                    else:
                        dst = kt[:, ct - NQH, tb * TB:(tb + 1) * TB]
                    nc.vector.tensor_copy(dst, ps[:])
                    # RoPE on this slice (DVE, overlaps next ct matmuls)
                    cs = cosf[:, tb * TB:(tb + 1) * TB]
                    sn = sinf[:, tb * TB:(tb + 1) * TB]
                    tmp = tmp_p.tile([128, TB], bf16, tag="tmp")
                    nc.vector.tensor_copy(tmp[0:64, :], dst[64:128, :])
                    nc.vector.tensor_copy(tmp[64:128, :], dst[0:64, :])
                    nc.vector.tensor_tensor(dst, dst, cs, mult)
                    nc.vector.tensor_tensor(tmp[:], tmp[:], sn, mult)
                    nc.vector.tensor_add(dst, dst, tmp[:])

                def do_v(st0, st1, tb, xch, first):
                    # V projection ([tok, d] layout)
                    if first and st0 == 0:
                        nc.sync.dma_start(wv[:], wv_d[:])
                    for st in range(st0, st1):
                        ps = psA.tile([128, NKVH * HD], f32, tag="ps")
                        for k in range(NKT):
                            nc.tensor.matmul(
                                ps[:],
                                xch[k // KCH][:, k % KCH, st * 128:(st + 1) * 128],
                                wv[:, k, :],
                                start=(k == 0), stop=(k == NKT - 1))
                        nc.vector.tensor_copy(vt[:, tb * (TB // 128) + st, :],
                                              ps[:])

                def attn_pair(j, pi):
                    nkv = (j + 1) * (TB // 128)
                    kvh = (2 * pi) // NREP
                    hh = (2 * pi, 2 * pi + 1)
                    po = []
                    for _i in range(2):
                        po_t = psO.tile([128, TB], f32, tag="po",
                                        name=f"po{_i}")
                        po.append(po_t)
                    gac = [None, None]
                    rrs = [None, None]

                    def emit_pv(pk, poff, pN, pts):
                        for i in range(2):
                            nc.tensor.matmul(
                                po[i][:, poff:],
                                vt[:, pk, kvh * HD:(kvh + 1) * HD],
                                pts[i][:, :pN],
                                start=(pk == 0), stop=(pk == nkv - 1),
                                skip_group_check=True)

                    pend = None
                    for k in range(nkv):
                        kd = k - j * (TB // 128)
                        off = max(0, kd) * 128   # causal col offset
                        N = TB - off
                        GSZ = 16                 # kv-tiles per sum group
                        gk = k % GSZ
                        glast = (gk == GSZ - 1) or (k == nkv - 1)
                        cur = []
                        for i, h in enumerate(hh):
                            # during attn(0) no wo blocks run, psW is idle:
                            # use it for h1 scores to relieve the psA cycle
                            if j == 0 and i == 1:
                                ss = psW.tile([128, TB], f32, tag="pw",
                                              name=f"ssb{pi}_{k}")
                            else:
                                ss = psA.tile([128, TB], f32, tag="ps")
                            nc.tensor.matmul(
                                ss[:, :N],
                                kt[:, kvh, k * 128:(k + 1) * 128],
                                qt[:, h, j * TB + off:(j + 1) * TB],
                                start=True, stop=True)
                            pt = pt_p.tile([128, TB], bf16, tag="pt")
                            nc.scalar.activation(pt[:, :N], ss[:, :N], Exp)
                            if kd >= 0:
                                # only the first 128 cols of the slice hold
                                # the causal triangle; the rest is all-ones
                                nc.vector.tensor_tensor(
                                    pt[:, :128], pt[:, :128],
                                    pat[:, kd, off:off + 128], mult)
                            cur.append(pt)
                            # ---- P row-sums: bf16 group accumulate on
                            # DVE, one ones-matmul per group of 4 ----
                            if gk == 0:
                                ga = ga_p.tile([128, TB], bf16, tag="ga")
                                gac[i] = ga
                                nc.vector.tensor_copy(ga[:, :N], pt[:, :N])
                            else:
                                nc.vector.tensor_add(
                                    gac[i][:, off:], gac[i][:, off:],
                                    pt[:, :N])
                        # depth-1 software pipeline: PV for k-1 lands
                        # after scores k, covering the exp+mask latency
                        if pend is not None:
                            emit_pv(*pend)
                        if k == 1:
                            emit_norm()   # previous pair, PE-covered
                        if glast:
                            for i in range(2):
                                nc.tensor.matmul(
                                    pmall[64 * i:64 * i + 1, :],
                                    ones[:], gac[i][:],
                                    start=(k < GSZ), stop=(k == nkv - 1),
                                    tile_position=(0, 64 * i),
                                    skip_group_check=True)
                            if k == nkv - 1:
                                # reciprocals ASAP so the rank-1
                                # broadcast matmuls aren't rr-blocked
                                for i in range(2):
                                    rr = rr_p.tile([1, TB], bf16,
                                                   tag="rr",
                                                   name=f"rr{pi}_{i}")
                                    rrs[i] = rr
                                    with nc.allow_low_precision(
                                            reason="1/sums bf16"):
                                        nc.vector.reciprocal(
                                            rr[:],
                                            pmall[64 * i:64 * i + 1, :])
                        # interleave wo fill work inside the k loop
                        if j > 0:
                            d0 = NDT * pi // (NQH // 2)
                            d1 = NDT * (pi + 1) // (NQH // 2)
                            nblk = d1 - d0
                            lo = nblk * k // nkv
                            hi = nblk * (k + 1) // nkv
                            for Dt in range(d0 + lo, d0 + hi):
                                wo_block(j - 1, Dt)
                        pend = (k, off, N, cur)
                    emit_pv(*pend)
                    pending[0] = (j, hh, po, rrs)

                for tb in range(NT):
                    # ---- projections + RoPE for token block tb, with the
                    # previous block's attention interleaved between units --
                    first = (_rep == 0 and tb == 0)
                    xch = []
                    w01 = [None, None]
                    if first:
                        # startup-optimized DMA order: feed the PE ASAP,
                        # defer constants it doesn't need yet
                        w0 = wqk_p.tile([128, NKT, 128], bf16, tag="wqk",
                                        name="w_first")
                        xc0 = xt_p.tile([128, KCH, TB], bf16, tag="xt")
                        NQ4 = NKT // 4
                        for q in range(4):
                            nc.sync.dma_start(
                                w0[:, q * NQ4:(q + 1) * NQ4, :],
                                wqk_d[NQH, :, q * NQ4:(q + 1) * NQ4, :])
                        nc.sync.dma_start(xc0[:, 0:KCH // 2, :],
                                          xt_d[0, :, 0:KCH // 2, :])
                        nc.sync.dma_start(xc0[:, KCH // 2:KCH, :],
                                          xt_d[0, :, KCH // 2:KCH, :])
                        w01[0] = w0
                        xch.append(xc0)
                        xc1 = xt_p.tile([128, KCH, TB], bf16, tag="xt")
                        nc.sync.dma_start(xc1[:], xt_d[0, :, KCH:2 * KCH, :])
                        xch.append(xc1)
                        w1 = wqk_p.tile([128, NKT, 128], bf16, tag="wqk",
                                        name="w_second")
                        nc.sync.dma_start(w1[:, 0:NKT // 2, :],
                                          wqk_d[NQH + 1, :, 0:NKT // 2, :])
                        nc.sync.dma_start(w1[:, NKT // 2:, :],
                                          wqk_d[NQH + 1, :, NKT // 2:, :])
                        w01[1] = w1
                        for ch in range(2, NCH):
                            xc = xt_p.tile([128, KCH, TB], bf16, tag="xt")
                            nc.sync.dma_start(
                                xc[:], xt_d[0, :, ch * KCH:(ch + 1) * KCH, :])
                            xch.append(xc)
                        nc.sync.dma_start(cosf[:], cosf_d[:])
                        nc.sync.dma_start(sinf[:], sinf_d[:])
                    else:
                        for ch in range(NCH):
                            xc = xt_p.tile([128, KCH, TB], bf16, tag="xt")
                            nc.sync.dma_start(
                                xc[:], xt_d[tb, :, ch * KCH:(ch + 1) * KCH, :])
                            xch.append(xc)

                    # pending norm of the previous attention's last pair
                    # must land before any wo block that reads its osb rows
                    emit_norm()

                    units = []
                    for ci, ct in enumerate([NQH, NQH + 1, 0, 1]):
                        units.append(
                            lambda ci=ci, ct=ct: do_ct(ci, ct, tb, xch,
                                                       w01, first))
                    units.append(lambda: do_v(0, 2, tb, xch, first))
                    units.append(lambda: do_v(2, 4, tb, xch, first))
                    for ci, ct in enumerate(range(2, NQH)):
                        units.append(
                            lambda ci=ci, ct=ct: do_ct(ci + 6, ct, tb, xch,
                                                       w01, first))

                    if tb == 0:
                        if first:
                            # pre-load the Exp activation table during the
                            # startup DMA window (LoadActFuncSet is ~1.3us)
                            nc.gpsimd.memset(warm[:], 0.0)
                            nc.scalar.activation(warm[:], warm[:], Exp)
                        for u in units:
                            u()
                        if first:
                            nc.sync.dma_start(pat[:], pat_d[:])
                            nc.sync.dma_start(ones[:], ones_d[:])
                            nc.sync.dma_start(ones2[:], ones2_d[:])
                    else:
                        # interleave attention(tb-1) pairs between proj units
                        seq = [[0, 1], [2, 3, 4], [5, 6, 7], [8, 9, 10], [11]]
                        for pi in range(NQH // 2):
                            for ui in seq[pi]:
                                units[ui]()
                            attn_pair(tb - 1, pi)
                        for ui in seq[NQH // 2]:
                            units[ui]()

                # ---- last attention block (wo(NT-2) interleaved inside) ----
                for pi in range(NQH // 2):
                    attn_pair(NT - 1, pi)

                # ---- tail: wo for the last q block ----
                emit_norm()
                for Dt in range(NDT):
                    wo_block(NJ - 1, Dt)

    nc.compile()
    return nc


_PERM = None


def _prep_core_inputs(x, freqs_cis, mask, wq, wk, wv, wo, b, g, S):
    """Host-side shard/permute/prepack for core (batch b, group g)."""
    global _PERM
    if _PERM is None or len(_PERM) != HD:
        _PERM = np.concatenate([np.arange(0, HD, 2), np.arange(1, HD, 2)])
    perm = _PERM
    NT = S // TB

    qh0 = g * NQH            # first q head
    kh0 = g * NKVH           # first kv head

    wq_g = wq[:, qh0 * HD:(qh0 + NQH) * HD].reshape(DIM, NQH, HD)[:, :, perm]
    wq_g = (wq_g * np.float32(HD ** -0.5)).reshape(DIM, NQH * HD)
    wk_g = wk[:, kh0 * HD:(kh0 + NKVH) * HD].reshape(DIM, NKVH, HD)[:, :, perm]
    wk_g = wk_g.reshape(DIM, NKVH * HD)
    wqk = np.concatenate([wq_g, wk_g], axis=1)          # [DIM, 1280]
    wqk = np.ascontiguousarray(
        wqk.reshape(NKT, 128, NCT, 128).transpose(2, 1, 0, 3)).astype(BF16)

    wv_g = wv[:, kh0 * HD:(kh0 + NKVH) * HD]            # [DIM, 256]
    wv_g = np.ascontiguousarray(
        wv_g.reshape(NKT, 128, NKVH * HD).transpose(1, 0, 2)).astype(BF16)

    wo_g = wo[qh0 * HD:(qh0 + NQH) * HD, :]             # [1024, DIM]
    wo_g = np.ascontiguousarray(
        wo_g.reshape(NQH, 128, DIM // 128, 128).transpose(2, 1, 0, 3)).astype(BF16)

    xb = x[b, :S, :]                                    # [S, DIM]
    xt = np.ascontiguousarray(
        xb.reshape(NT, TB, NKT, 128).transpose(0, 3, 2, 1)).astype(BF16)

    cos = freqs_cis[:S, :, 0]                           # [S, 64]
    sin = freqs_cis[:S, :, 1]
    cosf = np.ascontiguousarray(np.concatenate([cos, cos], 1).T).astype(BF16)
    sinf = np.ascontiguousarray(np.concatenate([-sin, sin], 1).T).astype(BF16)

    sub = mask[:TB, :TB]                                # [q, kv]
    pat = (sub.T.reshape(4, 128, TB) >= -0.5).astype(BF16)
    pat = np.ascontiguousarray(pat.transpose(1, 0, 2))

    ones = np.ones((128, 1), dtype=BF16)
    ones2 = np.ones((1, 128), dtype=BF16)
    return {"xt": xt, "wqk": wqk, "wv": wv_g, "wo": wo_g,
            "cosf": cosf, "sinf": sinf, "pat": pat, "ones": ones,
            "ones2": ones2}


def run(x, freqs_cis, mask, wq, wk, wv, wo, S=S_FULL, reps=1, time_it=False):
    from concourse.bass_utils import run_bass_kernel_spmd

    key = (S, reps)
    if key not in _cache:
        _cache[key] = _build(S, reps)
    nc = _cache[key]

    in_maps = []
    for c in range(NCORES):
        b, g = c // NGRP, c % NGRP
        in_maps.append(_prep_core_inputs(x, freqs_cis, mask, wq, wk, wv, wo,
                                         b, g, S))
    res = run_bass_kernel_spmd(nc, in_maps, core_ids=list(range(NCORES)))

    out = np.zeros((B, S, DIM), dtype=np.float32)
    for c in range(NCORES):
        b = c // NGRP
        pt = np.asarray(res.results[c]["outp"],
                        dtype=np.float32).reshape(DIM, S)  # partial final^T
        out[b] += pt.T
    return out


def kernel(x, start_pos, freqs_cis, mask, wq, wk, wv, wo):
    x = np.asarray(x, dtype=np.float32)
    freqs_cis = np.asarray(freqs_cis, dtype=np.float32)
    mask = np.asarray(mask, dtype=np.float32)
    wq = np.asarray(wq, dtype=np.float32)
    wk = np.asarray(wk, dtype=np.float32)
    wv = np.asarray(wv, dtype=np.float32)
    wo = np.asarray(wo, dtype=np.float32)
    return run(x, freqs_cis, mask, wq, wk, wv, wo, S=x.shape[1], reps=1)
